# revision 11
# baseline (speedup 1.0000x reference)
"""Trainium2 Bass kernel for the 5x5 Sinkhorn network (raw Bass, manual sync).

Reference computation (LENGTH=5, DIM=200, TEMP=0.01, 20 Sinkhorn iters):
    embs  = x[:,None] @ W_cont.T + b_cont          # [5,200]
    trans = embs @ W_in2.T + b_in2                 # [5,5]
    s     = trans / TEMP
    20x: s -= logsumexp(s, axis=0); s -= logsumexp(s, axis=1)
    out   = exp(s) @ x

Algebraic collapse used here (exact in fp32 up to rounding):
  1. The two linear layers collapse to an outer product:
         s[i,k] = (x_i * a_k + c_k + b2_k) / TEMP
     with a = W_in2 @ W_cont[:,0]  and  c = W_in2 @ b_cont.
  2. The log-space Sinkhorn iterations are equivalent to multiplicative
     scaling P = diag(u) K diag(v) with K = exp(s - colmax(s)):
         v = 1/(K^T u); u = 1/(K v)        (20 times, u0 = 1)
     and out = u * (K @ (v * x)).
  Each iteration is one tiny [5,5]x[5,1] matmul (PE) + one reciprocal (DVE);
  the chain is strictly serial, so sync is a simple per-engine counter.

Raw Bass (not Tile): the Tile context's exit sequence and the DVE
TensorTensorReduce instruction do not compile with the neuronxcc in this
environment, so semaphores are managed manually and reductions use
tensor_mul + reduce_sum.

Sharding: problem is far too small to shard; the kernel is replicated on
all 8 cores and core 0's output is returned.
"""

import numpy as np
from contextlib import ExitStack

import concourse.bass as bass
from concourse import mybir
from concourse.bass_utils import run_bass_kernel_spmd

L = 5
D = 200
N_SINKHORN = 20
INV_TEMP = 100.0  # 1 / 0.01

N_CORES = 8

_CACHE: dict = {}

Exp = mybir.ActivationFunctionType.Exp
Alu = mybir.AluOpType
Ax = mybir.AxisListType


def _bcast_rows(flat_ap, rows):
    # DRAM vector [N] read replicated into `rows` partitions -> [rows, N]
    return bass.AP(
        tensor=flat_ap.tensor,
        offset=flat_ap.offset,
        ap=[[0, rows]] + [list(d) for d in flat_ap.ap],
    )


def _build_nc() -> bass.Bass:
    nc = bass.Bass("TRN2")
    f32 = mybir.dt.float32

    x_d = nc.dram_tensor("x", [L], f32, kind="ExternalInput")
    wc_d = nc.dram_tensor("W_cont", [D, 1], f32, kind="ExternalInput")
    bc_d = nc.dram_tensor("b_cont", [D], f32, kind="ExternalInput")
    w2_d = nc.dram_tensor("W_in2", [L, D], f32, kind="ExternalInput")
    b2_d = nc.dram_tensor("b_in2", [L], f32, kind="ExternalInput")
    out_d = nc.dram_tensor("out", [L], f32, kind="ExternalOutput")

    N_DMA_SYNC = 4  # w2, b2(->ac3 col2), xcol, g3 row1   (HWDGE, dsem)
    N_DMA_SW = 2    # wc_b, bc_b                          (SWDGE, swsem)

    with ExitStack() as ctx:
        e = ctx.enter_context
        w2_sb = e(nc.sbuf_tensor("w2_sb", [L, D], f32))[:, :]
        wc_b = e(nc.sbuf_tensor("wc_b", [L, D], f32))[:, :]
        bc_b = e(nc.sbuf_tensor("bc_b", [L, D], f32))[:, :]
        scr = e(nc.sbuf_tensor("scr", [L, D], f32))[:, :]
        scr2 = e(nc.sbuf_tensor("scr2", [L, D], f32))[:, :]
        xcol = e(nc.sbuf_tensor("xcol", [L, 1], f32))[:, :]
        g3 = e(nc.sbuf_tensor("g3", [3, L], f32))[:, :]    # rows: ones, x, ones
        ident = e(nc.sbuf_tensor("ident", [L, L], f32))[:, :]
        ac3 = e(nc.sbuf_tensor("ac3", [L, 3], f32))[:, :]   # cols: c, a, b2
        acr = e(nc.sbuf_tensor("acr", [3, L], f32))[:, :]   # 100 * ac3^T
        ktsb = e(nc.sbuf_tensor("ktsb", [L, L], f32))[:, :]  # K^T
        ksb = e(nc.sbuf_tensor("ksb", [L, L], f32))[:, :]   # K
        negm = e(nc.sbuf_tensor("negm", [L, 1], f32))[:, :]
        warm = e(nc.sbuf_tensor("warm", [1, 1], f32))[:, :]
        ubuf = e(nc.sbuf_tensor("ubuf", [L, 1], f32))[:, :]
        vbuf = e(nc.sbuf_tensor("vbuf", [L, 1], f32))[:, :]
        acp = e(nc.psum_tensor("acp", [3, L], f32))[:, :]
        stp = e(nc.psum_tensor("stp", [L, L], f32))[:, :]
        kp = e(nc.psum_tensor("kp", [L, L], f32))[:, :]
        pvb = e(nc.psum_tensor("pvb", [L, 1], f32))[:, :]
        pub = e(nc.psum_tensor("pub", [L, 1], f32))[:, :]
        pfb = e(nc.psum_tensor("pfb", [L, 1], f32))[:, :]
        dsem = e(nc.semaphore(name="dsem"))    # HWDGE DMA completions (x16)
        swsem = e(nc.semaphore(name="swsem"))  # SWDGE DMA completions (x16)
        vsem = e(nc.semaphore(name="vsem"))    # DVE op count
        pesem = e(nc.semaphore(name="pesem"))  # PE op count
        asem = e(nc.semaphore(name="asem"))    # ACT op count
        psem = e(nc.semaphore(name="psem"))    # identity build steps
        wsem = e(nc.semaphore(name="wsem"))    # warm tile zeroed
        block = e(nc.Block())

        # DVE numbering (vsem after op):
        #  1 memset g3    2 memset ubuf  3 mul_c  4 red_c  5 mul_a  6 red_a
        #  7 acr_mul      8 negm         9 ksb copy
        #  8+2t recip v_t   9+2t recip u_t   (t = 1..20; u20 = 49)
        #  50 vx mul      51 out mul
        # PE numbering (pesem): 1 acp  2 stp  3 kp  2+2t pv_t  3+2t pu_t  44 pf

        @block.sync
        def _(sync):
            sync.dma_start(w2_sb, w2_d[:, :]).then_inc(dsem, 16)
            sync.dma_start(ac3[:, 2:3], b2_d[:, None]).then_inc(dsem, 16)
            sync.dma_start(xcol, x_d[:, None]).then_inc(dsem, 16)
            # g3 row1 = x; whole g3 memset to 1.0 by DVE op 1 first
            sync.wait_ge(vsem, 1)
            sync.dma_start(g3[1:2, :], x_d[None, :]).then_inc(dsem, 16)
            # output
            sync.wait_ge(vsem, 51)
            sync.dma_start(out_d[:, None], ubuf).then_inc(dsem, 16)
            sync.wait_ge(dsem, 16 * (N_DMA_SYNC + 1))

        @block.gpsimd
        def _(pool):
            pool.memset(warm, 0.0).then_inc(wsem, 1)
            pool.dma_start(wc_b, _bcast_rows(wc_d[:, 0], L)).then_inc(swsem, 16)
            pool.dma_start(bc_b, _bcast_rows(bc_d[:], L)).then_inc(swsem, 16)
            pool.memset(ident, 0.0).then_inc(psem, 1)
            pool.affine_select(
                out=ident, in_=ident,
                compare_op=Alu.not_equal, fill=1.0, base=0,
                pattern=[[-1, L]], channel_multiplier=1,
            ).wait_op(psem, 1, "sem-ge").then_inc(psem, 1)

        @block.scalar
        def _(act):
            # prewarm the Exp table while DMAs are in flight
            act.wait_ge(wsem, 1)
            nc.scalar.activation(warm, warm, Exp, bias=warm).then_inc(asem, 1)
            # KT = exp(ST100 - colmax)
            act.wait_ge(pesem, 2)
            act.wait_ge(vsem, 8)
            nc.scalar.activation(ktsb, stp, Exp, bias=negm).then_inc(asem, 1)

        @block.vector
        def _(vec):
            vec.memset(g3, 1.0).then_inc(vsem, 1)                          # 1
            vec.memset(ubuf, 1.0).then_inc(vsem, 1)                        # 2
            vec.wait_ge(dsem, 16 * N_DMA_SYNC)
            vec.wait_ge(swsem, 16 * N_DMA_SW)
            nc.vector.tensor_mul(scr, w2_sb, bc_b).then_inc(vsem, 1)       # 3
            nc.vector.reduce_sum(ac3[:, 0:1], scr, axis=Ax.X) \
                .wait_op(vsem, 3, "sem-ge").then_inc(vsem, 1)              # 4: c
            nc.vector.tensor_mul(scr2, w2_sb, wc_b).then_inc(vsem, 1)      # 5
            nc.vector.reduce_sum(ac3[:, 1:2], scr2, axis=Ax.X) \
                .wait_op(vsem, 5, "sem-ge").then_inc(vsem, 1)              # 6: a
            nc.vector.tensor_scalar_mul(acr, acp, INV_TEMP) \
                .wait_op(pesem, 1, "sem-ge").then_inc(vsem, 1)             # 7
            nc.vector.reduce_max(negm, stp, axis=Ax.X, negate=True) \
                .wait_op(pesem, 2, "sem-ge").then_inc(vsem, 1)             # 8
            nc.vector.tensor_copy(ksb, kp) \
                .wait_op(pesem, 3, "sem-ge").then_inc(vsem, 1)             # 9
            for t in range(1, N_SINKHORN + 1):
                nc.vector.reciprocal(vbuf, pvb) \
                    .wait_op(pesem, 2 + 2 * t, "sem-ge").then_inc(vsem, 1)
                nc.vector.reciprocal(ubuf, pub) \
                    .wait_op(pesem, 3 + 2 * t, "sem-ge").then_inc(vsem, 1)
            nc.vector.tensor_mul(vbuf, vbuf, xcol) \
                .wait_op(vsem, 7 + 2 * N_SINKHORN + 1, "sem-ge").then_inc(vsem, 1)  # 50
            vec.wait_ge(vsem, 9 + 2 * N_SINKHORN)
            nc.vector.tensor_mul(ubuf, pfb, ubuf) \
                .wait_op(pesem, 4 + 2 * N_SINKHORN, "sem-ge").then_inc(vsem, 1)  # 51

        @block.tensor
        def _(pe):
            pe.wait_ge(vsem, 6)
            pe.wait_ge(psem, 2)
            nc.tensor.matmul(acp, ac3, ident, start=True, stop=True) \
                .then_inc(pesem, 1)                                        # 1
            nc.tensor.matmul(stp, acr, g3, start=True, stop=True) \
                .wait_op(vsem, 7, "sem-ge").then_inc(pesem, 1)             # 2: ST100
            nc.tensor.matmul(kp, ktsb, ident, start=True, stop=True) \
                .wait_op(asem, 2, "sem-ge").then_inc(pesem, 1)             # 3: K
            for t in range(1, N_SINKHORN + 1):
                # pv_t = K^T u_{t-1}
                nc.tensor.matmul(pvb, ksb, ubuf, start=True, stop=True) \
                    .wait_op(vsem, 7 + 2 * t, "sem-ge").then_inc(pesem, 1)
                # pu_t = K v_t
                nc.tensor.matmul(pub, ktsb, vbuf, start=True, stop=True) \
                    .wait_op(vsem, 8 + 2 * t, "sem-ge").then_inc(pesem, 1)
            nc.tensor.matmul(pfb, ktsb, vbuf, start=True, stop=True) \
                .wait_op(vsem, 50, "sem-ge").then_inc(pesem, 1)            # 44

    return nc


def _get_nc() -> bass.Bass:
    if "nc" not in _CACHE:
        _CACHE["nc"] = _build_nc()
    return _CACHE["nc"]


def kernel(**inputs: np.ndarray) -> np.ndarray:
    nc = _get_nc()
    in_map = {
        "x": np.ascontiguousarray(np.asarray(inputs["x"], dtype=np.float32)),
        "W_cont": np.ascontiguousarray(np.asarray(inputs["W_cont"], dtype=np.float32)),
        "b_cont": np.ascontiguousarray(np.asarray(inputs["b_cont"], dtype=np.float32)),
        "W_in2": np.ascontiguousarray(np.asarray(inputs["W_in2"], dtype=np.float32)),
        "b_in2": np.ascontiguousarray(np.asarray(inputs["b_in2"], dtype=np.float32)),
    }
    res = run_bass_kernel_spmd(
        nc, [dict(in_map) for _ in range(N_CORES)], core_ids=list(range(N_CORES))
    )
    return np.asarray(res.results[0]["out"], dtype=np.float32)


# revision 14
# speedup vs baseline: 1.0449x; 1.0449x over previous
"""Trainium2 Bass kernel for the 5x5 Sinkhorn network (raw Bass, manual sync).

Reference computation (LENGTH=5, DIM=200, TEMP=0.01, 20 Sinkhorn iters):
    embs  = x[:,None] @ W_cont.T + b_cont          # [5,200]
    trans = embs @ W_in2.T + b_in2                 # [5,5]
    s     = trans / TEMP
    20x: s -= logsumexp(s, axis=0); s -= logsumexp(s, axis=1)
    out   = exp(s) @ x

Algebraic collapse used here (exact in fp32 up to rounding):
  1. The two linear layers collapse to an outer product:
         s[i,k] = (x_i * a_k + c_k + b2_k) / TEMP
     with a = W_in2 @ W_cont[:,0]  and  c = W_in2 @ b_cont.
  2. The log-space Sinkhorn iterations are equivalent to multiplicative
     scaling P = diag(u) K diag(v) with K = exp(s - colmax(s)):
         v = 1/(K^T u); u = 1/(K v)        (20 times, u0 = 1)
     and out = u * (K @ (v * x)).
  Each iteration is one tiny [5,5]x[5,1] matmul (PE) + one reciprocal (DVE);
  the chain is strictly serial, so sync is per-engine op counters.
  v_1 = 1/(K^T 1) comes for free from the Exp activation's accum_out
  (row sums of K^T), skipping the first matmul.

Raw Bass (not Tile): the Tile context's exit sequence and the DVE
TensorTensorReduce instruction do not compile with the neuronxcc in this
environment, so semaphores are managed manually.

Sharding: problem is far too small to shard; the kernel is replicated on
all 8 cores and core 0's output is returned.
"""

import numpy as np
from contextlib import ExitStack

import concourse.bass as bass
from concourse import mybir
from concourse.bass_utils import run_bass_kernel_spmd

L = 5
D = 200
N_SINKHORN = 20
INV_TEMP = 100.0  # 1 / 0.01

N_CORES = 8

_CACHE: dict = {}

Exp = mybir.ActivationFunctionType.Exp
Alu = mybir.AluOpType
Ax = mybir.AxisListType


def _bcast_rows(flat_ap, rows):
    # DRAM vector [N] read replicated into `rows` partitions -> [rows, N]
    return bass.AP(
        tensor=flat_ap.tensor,
        offset=flat_ap.offset,
        ap=[[0, rows]] + [list(d) for d in flat_ap.ap],
    )


def _build_nc() -> bass.Bass:
    nc = bass.Bass("TRN2")
    f32 = mybir.dt.float32

    x_d = nc.dram_tensor("x", [L], f32, kind="ExternalInput")
    wc_d = nc.dram_tensor("W_cont", [D, 1], f32, kind="ExternalInput")
    bc_d = nc.dram_tensor("b_cont", [D], f32, kind="ExternalInput")
    w2_d = nc.dram_tensor("W_in2", [L, D], f32, kind="ExternalInput")
    b2_d = nc.dram_tensor("b_in2", [L], f32, kind="ExternalInput")
    out_d = nc.dram_tensor("out", [L], f32, kind="ExternalOutput")

    with ExitStack() as ctx:
        e = ctx.enter_context
        w2_sb = e(nc.sbuf_tensor("w2_sb", [L, D], f32))[:, :]
        wc_b = e(nc.sbuf_tensor("wc_b", [L, D], f32))[:, :]
        bc_b = e(nc.sbuf_tensor("bc_b", [L, D], f32))[:, :]
        scr = e(nc.sbuf_tensor("scr", [L, 2 * D], f32))[:, :]
        g3 = e(nc.sbuf_tensor("g3", [3, L], f32))[:, :]     # rows: x, ones, ones
        ident = e(nc.sbuf_tensor("ident", [L, L], f32))[:, :]
        ac3 = e(nc.sbuf_tensor("ac3", [L, 3], f32))[:, :]   # cols: a, c, b2
        acr = e(nc.sbuf_tensor("acr", [3, L], f32))[:, :]   # 100 * ac3^T
        ktsb = e(nc.sbuf_tensor("ktsb", [L, L], f32))[:, :]  # K^T
        ksb = e(nc.sbuf_tensor("ksb", [L, L], f32))[:, :]   # K
        negm = e(nc.sbuf_tensor("negm", [L, 1], f32))[:, :]
        warm = e(nc.sbuf_tensor("warm", [1, 1], f32))[:, :]
        onecol = e(nc.sbuf_tensor("onecol", [1, 1], f32))[:, :]
        pv1acc = e(nc.sbuf_tensor("pv1acc", [L, 1], f32))[:, :]  # K^T @ 1
        ubuf = e(nc.sbuf_tensor("ubuf", [L, 1], f32))[:, :]
        vbuf = e(nc.sbuf_tensor("vbuf", [L, 1], f32))[:, :]
        acp = e(nc.psum_tensor("acp", [3, L], f32))[:, :]
        stp = e(nc.psum_tensor("stp", [L, L], f32))[:, :]
        kp = e(nc.psum_tensor("kp", [L, L], f32))[:, :]
        pvb = e(nc.psum_tensor("pvb", [L, 1], f32))[:, :]
        pub = e(nc.psum_tensor("pub", [L, 1], f32))[:, :]
        pfb = e(nc.psum_tensor("pfb", [L, 1], f32))[:, :]
        xp = e(nc.psum_tensor("xp", [L, 1], f32))[:, :]     # x as a column
        dsem = e(nc.semaphore(name="dsem"))   # HWDGE DMA completions (x16)
        gsem = e(nc.semaphore(name="gsem"))   # g3 row1 (x) DMA completion
        vsem = e(nc.semaphore(name="vsem"))   # DVE op count
        pesem = e(nc.semaphore(name="pesem"))  # PE op count
        asem = e(nc.semaphore(name="asem"))   # ACT op count
        psem = e(nc.semaphore(name="psem"))   # identity build steps
        block = e(nc.Block())

        # --- DVE op indices (vsem value after each) ---
        V_MS_WARM = 1
        V_MS_G3 = 2
        V_MS_UBUF = 3
        V_MS_ONE = 4
        V_MUL_A = 5
        V_MUL_C = 6
        V_RED_A = 7
        V_RED_C = 8
        V_ACR = 9
        V_NEGM = 10
        V_V1 = 11
        V_KSB = 12
        V_U1 = 13
        def V_V(t):  # t >= 2
            return 10 + 2 * t
        def V_U(t):  # t >= 2
            return 11 + 2 * t
        V_VX = V_U(N_SINKHORN) + 1      # 51
        V_OUT = V_VX + 1                # 52

        # --- PE op indices (pesem value after each) ---
        P_ACP = 1
        P_STP = 2
        P_KP = 3
        P_PU1 = 4
        def P_PV(t):  # t >= 2
            return 1 + 2 * t
        def P_PU(t):  # t >= 2
            return 2 + 2 * t
        P_XP = P_PU(N_SINKHORN) + 1     # 43
        P_PF = P_XP + 1                 # 44

        N_DSEM = 16 * 5  # w2, bc_b, b2, wc_b, out

        @block.sync
        def _(sync):
            sync.dma_start(w2_sb, w2_d[:, :]).then_inc(dsem, 16)
            sync.dma_start(bc_b, _bcast_rows(bc_d[:], L)).then_inc(dsem, 16)
            sync.wait_ge(vsem, V_MS_G3)
            sync.dma_start(g3[0:1, :], x_d[None, :]).then_inc(gsem, 16)
            sync.wait_ge(vsem, V_OUT)
            sync.dma_start(out_d[:, None], ubuf).then_inc(dsem, 16)
            sync.wait_ge(dsem, N_DSEM)

        @block.scalar
        def _(act):
            nc.scalar.dma_start(ac3[:, 2:3], b2_d[:, None]).then_inc(dsem, 16)
            nc.scalar.dma_start(wc_b, _bcast_rows(wc_d[:, 0], L)).then_inc(dsem, 16)
            # prewarm the Exp table while DMAs complete
            act.wait_ge(vsem, V_MS_WARM)
            nc.scalar.activation(warm, warm, Exp, bias=warm).then_inc(asem, 1)
            # KT = exp(ST100 - colmax); accum_out = row sums of KT = K^T @ 1 = 1/v_1
            act.wait_ge(pesem, P_STP)
            act.wait_ge(vsem, V_NEGM)
            nc.scalar.activation(
                ktsb, stp, Exp, bias=negm, accum_out=pv1acc
            ).then_inc(asem, 1)

        @block.gpsimd
        def _(pool):
            pool.memset(ident, 0.0).then_inc(psem, 1)
            pool.affine_select(
                out=ident, in_=ident,
                compare_op=Alu.not_equal, fill=1.0, base=0,
                pattern=[[-1, L]], channel_multiplier=1,
            ).wait_op(psem, 1, "sem-ge").then_inc(psem, 1)

        @block.vector
        def _(vec):
            vec.memset(warm, 0.0).then_inc(vsem, 1)                         # 1
            vec.memset(g3, 1.0).then_inc(vsem, 1)                           # 2
            vec.memset(ubuf, 1.0).then_inc(vsem, 1)                         # 3
            vec.memset(onecol, 1.0).then_inc(vsem, 1)                       # 4
            vec.wait_ge(dsem, 16 * 4)  # w2, bc_b, b2, wc_b
            nc.vector.tensor_mul(scr[:, 0:D], w2_sb, wc_b).then_inc(vsem, 1)    # 5: a
            nc.vector.tensor_mul(scr[:, D:2 * D], w2_sb, bc_b).then_inc(vsem, 1)  # 6: c
            nc.vector.reduce_sum(ac3[:, 0:1], scr[:, 0:D], axis=Ax.X) \
                .wait_op(vsem, V_MUL_A, "sem-ge").then_inc(vsem, 1)         # 7
            nc.vector.reduce_sum(ac3[:, 1:2], scr[:, D:2 * D], axis=Ax.X) \
                .wait_op(vsem, V_MUL_C, "sem-ge").then_inc(vsem, 1)         # 8
            nc.vector.tensor_scalar_mul(acr, acp, INV_TEMP) \
                .wait_op(pesem, P_ACP, "sem-ge").then_inc(vsem, 1)          # 8
            nc.vector.reduce_max(negm, stp, axis=Ax.X, negate=True) \
                .wait_op(pesem, P_STP, "sem-ge").then_inc(vsem, 1)          # 9
            nc.vector.reciprocal(vbuf, pv1acc) \
                .wait_op(asem, 2, "sem-ge").then_inc(vsem, 1)               # 10: v_1
            nc.vector.tensor_copy(ksb, kp) \
                .wait_op(pesem, P_KP, "sem-ge").then_inc(vsem, 1)           # 11
            nc.vector.reciprocal(ubuf, pub) \
                .wait_op(pesem, P_PU1, "sem-ge").then_inc(vsem, 1)          # 12: u_1
            for t in range(2, N_SINKHORN + 1):
                nc.vector.reciprocal(vbuf, pvb) \
                    .wait_op(pesem, P_PV(t), "sem-ge").then_inc(vsem, 1)
                nc.vector.reciprocal(ubuf, pub) \
                    .wait_op(pesem, P_PU(t), "sem-ge").then_inc(vsem, 1)
            vec.wait_ge(vsem, V_V(N_SINKHORN))  # vbuf write (2 ops back) landed
            nc.vector.tensor_mul(vbuf, vbuf, xp) \
                .wait_op(pesem, P_XP, "sem-ge").then_inc(vsem, 1)           # vx
            vec.wait_ge(vsem, V_U(N_SINKHORN))  # ubuf write landed
            nc.vector.tensor_mul(ubuf, pfb, ubuf) \
                .wait_op(pesem, P_PF, "sem-ge").then_inc(vsem, 1)           # out

        @block.tensor
        def _(pe):
            pe.wait_ge(vsem, V_RED_C)
            pe.wait_ge(psem, 2)
            nc.tensor.matmul(acp, ac3, ident, start=True, stop=True) \
                .then_inc(pesem, 1)                                         # acp3
            pe.wait_ge(gsem, 16)
            nc.tensor.matmul(stp, acr, g3, start=True, stop=True) \
                .wait_op(vsem, V_ACR, "sem-ge").then_inc(pesem, 1)          # ST100
            nc.tensor.matmul(kp, ktsb, ident, start=True, stop=True) \
                .wait_op(asem, 2, "sem-ge").then_inc(pesem, 1)              # K
            nc.tensor.matmul(pub, ktsb, vbuf, start=True, stop=True) \
                .wait_op(vsem, V_V1, "sem-ge").then_inc(pesem, 1)           # pu_1
            for t in range(2, N_SINKHORN + 1):
                nc.tensor.matmul(pvb, ksb, ubuf, start=True, stop=True) \
                    .wait_op(vsem, V_U(t - 1), "sem-ge").then_inc(pesem, 1)
                nc.tensor.matmul(pub, ktsb, vbuf, start=True, stop=True) \
                    .wait_op(vsem, V_V(t), "sem-ge").then_inc(pesem, 1)
            # x as a column (for the epilogue), via a K=1 matmul on g3 row 1
            nc.tensor.matmul(xp, g3[0:1, :], onecol, start=True, stop=True) \
                .then_inc(pesem, 1)                                         # xp
            nc.tensor.matmul(pfb, ktsb, vbuf, start=True, stop=True) \
                .wait_op(vsem, V_VX, "sem-ge").then_inc(pesem, 1)           # pf

    return nc


def _get_nc() -> bass.Bass:
    if "nc" not in _CACHE:
        _CACHE["nc"] = _build_nc()
    return _CACHE["nc"]


def kernel(**inputs: np.ndarray) -> np.ndarray:
    nc = _get_nc()
    in_map = {
        "x": np.ascontiguousarray(np.asarray(inputs["x"], dtype=np.float32)),
        "W_cont": np.ascontiguousarray(np.asarray(inputs["W_cont"], dtype=np.float32)),
        "b_cont": np.ascontiguousarray(np.asarray(inputs["b_cont"], dtype=np.float32)),
        "W_in2": np.ascontiguousarray(np.asarray(inputs["W_in2"], dtype=np.float32)),
        "b_in2": np.ascontiguousarray(np.asarray(inputs["b_in2"], dtype=np.float32)),
    }
    res = run_bass_kernel_spmd(
        nc, [dict(in_map) for _ in range(N_CORES)], core_ids=list(range(N_CORES))
    )
    return np.asarray(res.results[0]["out"], dtype=np.float32)


# revision 17
# speedup vs baseline: 1.0691x; 1.0231x over previous
"""Trainium2 Bass kernel for the 5x5 Sinkhorn network (raw Bass, manual sync).

Reference computation (LENGTH=5, DIM=200, TEMP=0.01, 20 Sinkhorn iters):
    embs  = x[:,None] @ W_cont.T + b_cont          # [5,200]
    trans = embs @ W_in2.T + b_in2                 # [5,5]
    s     = trans / TEMP
    20x: s -= logsumexp(s, axis=0); s -= logsumexp(s, axis=1)
    out   = exp(s) @ x

Algebraic collapse used here (exact in fp32 up to rounding):
  1. The two linear layers collapse to an outer product:
         s[i,k] = (x_i * a_k + c_k + b2_k) / TEMP
     with a = W_in2 @ W_cont[:,0]  and  c = W_in2 @ b_cont.
  2. The log-space Sinkhorn iterations are equivalent to multiplicative
     scaling P = diag(u) K diag(v) with K = exp(s - colmax(s)):
         v = 1/(K^T u); u = 1/(K v)        (20 times, u0 = 1)
     and out = u * (K @ (v * x)).
  Each iteration is one tiny [5,5]x[5,1] matmul (PE) + one reciprocal (DVE);
  the chain is strictly serial, so sync is per-engine op counters.
  v_1 = 1/(K^T 1) comes for free from the Exp activation's accum_out
  (row sums of K^T), skipping the first matmul.

Raw Bass (not Tile): the Tile context's exit sequence and the DVE
TensorTensorReduce instruction do not compile with the neuronxcc in this
environment, so semaphores are managed manually.

Sharding: problem is far too small to shard; the kernel is replicated on
all 8 cores and core 0's output is returned.
"""

import numpy as np
from contextlib import ExitStack

import concourse.bass as bass
from concourse import mybir
from concourse.bass_utils import run_bass_kernel_spmd

L = 5
D = 200
N_SINKHORN = 20
INV_TEMP = 100.0  # 1 / 0.01

N_CORES = 8

_CACHE: dict = {}

Exp = mybir.ActivationFunctionType.Exp
Alu = mybir.AluOpType
Ax = mybir.AxisListType


def _bcast_rows(flat_ap, rows):
    # DRAM vector [N] read replicated into `rows` partitions -> [rows, N]
    return bass.AP(
        tensor=flat_ap.tensor,
        offset=flat_ap.offset,
        ap=[[0, rows]] + [list(d) for d in flat_ap.ap],
    )


def _build_nc() -> bass.Bass:
    nc = bass.Bass("TRN2")
    f32 = mybir.dt.float32

    x_d = nc.dram_tensor("x", [L], f32, kind="ExternalInput")
    wc_d = nc.dram_tensor("W_cont", [D, 1], f32, kind="ExternalInput")
    bc_d = nc.dram_tensor("b_cont", [D], f32, kind="ExternalInput")
    w2_d = nc.dram_tensor("W_in2", [L, D], f32, kind="ExternalInput")
    b2_d = nc.dram_tensor("b_in2", [L], f32, kind="ExternalInput")
    out_d = nc.dram_tensor("out", [L], f32, kind="ExternalOutput")

    with ExitStack() as ctx:
        e = ctx.enter_context
        w2_sb = e(nc.sbuf_tensor("w2_sb", [L, D], f32))[:, :]
        wc_b = e(nc.sbuf_tensor("wc_b", [L, D], f32))[:, :]
        bc_b = e(nc.sbuf_tensor("bc_b", [L, D], f32))[:, :]
        scr = e(nc.sbuf_tensor("scr", [L, 2 * D], f32))[:, :]
        g3 = e(nc.sbuf_tensor("g3", [3, L], f32))[:, :]     # rows: x, ones, 100
        ident = e(nc.sbuf_tensor("ident", [L, L], f32))[:, :]
        ac2 = e(nc.sbuf_tensor("ac2", [L, 2], f32))[:, :]   # cols: a, c
        acr = e(nc.sbuf_tensor("acr", [3, L], f32))[:, :]   # 100a, 100c, b2
        ktsb = e(nc.sbuf_tensor("ktsb", [L, L], f32))[:, :]  # K^T
        ksb = e(nc.sbuf_tensor("ksb", [L, L], f32))[:, :]   # K
        negm = e(nc.sbuf_tensor("negm", [L, 1], f32))[:, :]
        warm = e(nc.sbuf_tensor("warm", [1, 1], f32))[:, :]
        onecol = e(nc.sbuf_tensor("onecol", [1, 1], f32))[:, :]
        pv1acc = e(nc.sbuf_tensor("pv1acc", [L, 1], f32))[:, :]  # K^T @ 1
        ubuf = e(nc.sbuf_tensor("ubuf", [L, 1], f32))[:, :]
        vbuf = e(nc.sbuf_tensor("vbuf", [L, 1], f32))[:, :]
        acp = e(nc.psum_tensor("acp", [2, L], f32))[:, :]
        stp = e(nc.psum_tensor("stp", [L, L], f32))[:, :]
        kp = e(nc.psum_tensor("kp", [L, L], f32))[:, :]
        pvb = e(nc.psum_tensor("pvb", [L, 1], f32))[:, :]
        pub = e(nc.psum_tensor("pub", [L, 1], f32))[:, :]
        pfb = e(nc.psum_tensor("pfb", [L, 1], f32))[:, :]
        xp = e(nc.psum_tensor("xp", [L, 1], f32))[:, :]     # x as a column
        dsem = e(nc.semaphore(name="dsem"))   # HWDGE DMA completions (x16)
        gsem = e(nc.semaphore(name="gsem"))   # g3 row1 (x) DMA completion
        vsem = e(nc.semaphore(name="vsem"))   # DVE op count
        pesem = e(nc.semaphore(name="pesem"))  # PE op count
        asem = e(nc.semaphore(name="asem"))   # ACT op count
        psem = e(nc.semaphore(name="psem"))   # identity build steps
        block = e(nc.Block())

        # --- DVE op indices (vsem value after each) ---
        V_MS_WARM = 1
        V_MS_G3A = 2
        V_MS_G3B = 3
        V_MS_UBUF = 4
        V_MS_ONE = 5
        V_MUL_A = 6
        V_MUL_C = 7
        V_RED_A = 8
        V_RED_C = 9
        V_ACR = 10
        V_NEGM = 11
        V_V1 = 12
        V_KSB = 13
        V_U1 = 14
        def V_V(t):  # t >= 2
            return 11 + 2 * t
        def V_U(t):  # t >= 2
            return 12 + 2 * t
        V_VX = V_U(N_SINKHORN) + 1      # 51
        V_OUT = V_VX + 1                # 52

        # --- PE op indices (pesem value after each) ---
        P_ACP = 1
        P_STP = 2
        P_KP = 3
        P_PU1 = 4
        def P_PV(t):  # t >= 2
            return 1 + 2 * t
        def P_PU(t):  # t >= 2
            return 2 + 2 * t
        P_XP = P_PU(N_SINKHORN) + 1     # 43
        P_PF = P_XP + 1                 # 44

        N_DSEM = 16 * 4  # w2, bc_b, wc_b, out

        @block.sync
        def _(sync):
            sync.dma_start(w2_sb, w2_d[:, :]).then_inc(dsem, 16)
            sync.dma_start(bc_b, _bcast_rows(bc_d[:], L)).then_inc(dsem, 16)
            sync.wait_ge(vsem, V_MS_G3B)
            sync.dma_start(g3[0:1, :], x_d[None, :]).then_inc(gsem, 16)
            sync.dma_start(acr[2:3, :], b2_d[None, :]).then_inc(gsem, 16)
            sync.wait_ge(vsem, V_OUT)
            sync.dma_start(out_d[:, None], ubuf).then_inc(dsem, 16)
            sync.wait_ge(dsem, N_DSEM)

        @block.scalar
        def _(act):
            nc.scalar.dma_start(wc_b, _bcast_rows(wc_d[:, 0], L)).then_inc(dsem, 16)
            # prewarm the Exp table while DMAs complete
            act.wait_ge(vsem, V_MS_WARM)
            nc.scalar.activation(warm, warm, Exp, bias=warm).then_inc(asem, 1)
            # KT = exp(ST100 - colmax); accum_out = row sums of KT = K^T @ 1 = 1/v_1
            act.wait_ge(pesem, P_STP)
            act.wait_ge(vsem, V_NEGM)
            nc.scalar.activation(
                ktsb, stp, Exp, bias=negm, accum_out=pv1acc
            ).then_inc(asem, 1)

        @block.gpsimd
        def _(pool):
            pool.memset(ident, 0.0).then_inc(psem, 1)
            pool.affine_select(
                out=ident, in_=ident,
                compare_op=Alu.not_equal, fill=1.0, base=0,
                pattern=[[-1, L]], channel_multiplier=1,
            ).wait_op(psem, 1, "sem-ge").then_inc(psem, 1)

        @block.vector
        def _(vec):
            vec.memset(warm, 0.0).then_inc(vsem, 1)                         # 1
            vec.memset(g3, INV_TEMP).then_inc(vsem, 1)                      # 2
            vec.memset(g3[0:2, :], 1.0) \
                .wait_op(vsem, 2, "sem-ge").then_inc(vsem, 1)               # 3
            vec.memset(ubuf, 1.0).then_inc(vsem, 1)                         # 4
            vec.memset(onecol, 1.0).then_inc(vsem, 1)                       # 5
            vec.wait_ge(dsem, 16 * 3)  # w2, bc_b, wc_b
            nc.vector.tensor_mul(scr[:, 0:D], w2_sb, wc_b).then_inc(vsem, 1)    # 5: a
            nc.vector.tensor_mul(scr[:, D:2 * D], w2_sb, bc_b).then_inc(vsem, 1)  # 6: c
            nc.vector.reduce_sum(ac2[:, 0:1], scr[:, 0:D], axis=Ax.X) \
                .wait_op(vsem, V_MUL_A, "sem-ge").then_inc(vsem, 1)         # red_a
            nc.vector.reduce_sum(ac2[:, 1:2], scr[:, D:2 * D], axis=Ax.X) \
                .wait_op(vsem, V_MUL_C, "sem-ge").then_inc(vsem, 1)         # red_c
            nc.vector.tensor_scalar_mul(acr[0:2, :], acp, INV_TEMP) \
                .wait_op(pesem, P_ACP, "sem-ge").then_inc(vsem, 1)          # acr
            nc.vector.reduce_max(negm, stp, axis=Ax.X, negate=True) \
                .wait_op(pesem, P_STP, "sem-ge").then_inc(vsem, 1)          # 9
            nc.vector.reciprocal(vbuf, pv1acc) \
                .wait_op(asem, 2, "sem-ge").then_inc(vsem, 1)               # 10: v_1
            nc.vector.tensor_copy(ksb, kp) \
                .wait_op(pesem, P_KP, "sem-ge").then_inc(vsem, 1)           # 11
            nc.vector.reciprocal(ubuf, pub) \
                .wait_op(pesem, P_PU1, "sem-ge").then_inc(vsem, 1)          # 12: u_1
            for t in range(2, N_SINKHORN + 1):
                nc.vector.reciprocal(vbuf, pvb) \
                    .wait_op(pesem, P_PV(t), "sem-ge").then_inc(vsem, 1)
                nc.vector.reciprocal(ubuf, pub) \
                    .wait_op(pesem, P_PU(t), "sem-ge").then_inc(vsem, 1)
            vec.wait_ge(vsem, V_V(N_SINKHORN))  # vbuf write (2 ops back) landed
            nc.vector.tensor_mul(vbuf, vbuf, xp) \
                .wait_op(pesem, P_XP, "sem-ge").then_inc(vsem, 1)           # vx
            vec.wait_ge(vsem, V_U(N_SINKHORN))  # ubuf write landed
            nc.vector.tensor_mul(ubuf, pfb, ubuf) \
                .wait_op(pesem, P_PF, "sem-ge").then_inc(vsem, 1)           # out

        @block.tensor
        def _(pe):
            pe.wait_ge(vsem, V_RED_C)
            pe.wait_ge(psem, 2)
            nc.tensor.matmul(acp, ac2, ident, start=True, stop=True) \
                .then_inc(pesem, 1)                                         # acp2
            pe.wait_ge(gsem, 32)
            nc.tensor.matmul(stp, acr, g3, start=True, stop=True) \
                .wait_op(vsem, V_ACR, "sem-ge").then_inc(pesem, 1)          # ST100
            nc.tensor.matmul(kp, ktsb, ident, start=True, stop=True) \
                .wait_op(asem, 2, "sem-ge").then_inc(pesem, 1)              # K
            nc.tensor.matmul(pub, ktsb, vbuf, start=True, stop=True) \
                .wait_op(vsem, V_V1, "sem-ge").then_inc(pesem, 1)           # pu_1
            for t in range(2, N_SINKHORN + 1):
                nc.tensor.matmul(pvb, ksb, ubuf, start=True, stop=True) \
                    .wait_op(vsem, V_U(t - 1), "sem-ge").then_inc(pesem, 1)
                nc.tensor.matmul(pub, ktsb, vbuf, start=True, stop=True) \
                    .wait_op(vsem, V_V(t), "sem-ge").then_inc(pesem, 1)
            # x as a column (for the epilogue), via a K=1 matmul on g3 row 1
            nc.tensor.matmul(xp, g3[0:1, :], onecol, start=True, stop=True) \
                .then_inc(pesem, 1)                                         # xp
            nc.tensor.matmul(pfb, ktsb, vbuf, start=True, stop=True) \
                .wait_op(vsem, V_VX, "sem-ge").then_inc(pesem, 1)           # pf

    return nc


def _get_nc() -> bass.Bass:
    if "nc" not in _CACHE:
        _CACHE["nc"] = _build_nc()
    return _CACHE["nc"]


def kernel(**inputs: np.ndarray) -> np.ndarray:
    nc = _get_nc()
    in_map = {
        "x": np.ascontiguousarray(np.asarray(inputs["x"], dtype=np.float32)),
        "W_cont": np.ascontiguousarray(np.asarray(inputs["W_cont"], dtype=np.float32)),
        "b_cont": np.ascontiguousarray(np.asarray(inputs["b_cont"], dtype=np.float32)),
        "W_in2": np.ascontiguousarray(np.asarray(inputs["W_in2"], dtype=np.float32)),
        "b_in2": np.ascontiguousarray(np.asarray(inputs["b_in2"], dtype=np.float32)),
    }
    res = run_bass_kernel_spmd(
        nc, [dict(in_map) for _ in range(N_CORES)], core_ids=list(range(N_CORES))
    )
    return np.asarray(res.results[0]["out"], dtype=np.float32)
